# revision 1
# baseline (speedup 1.0000x reference)
"""Deformable attention TRN2 kernel (nn_DeformAttn).

Self-contained: host prep + Bass/Tile device program + 8-core SPMD launch.
Core 2*b+half computes output rows [24*half, 24*half+24) of batch b.
"""
import sys
from contextlib import ExitStack

import numpy as np

if "/opt/trn_rl_repo" not in sys.path:
    sys.path.insert(0, "/opt/trn_rl_repo")

import concourse.bass as bass
import concourse.tile as tile
from concourse import mybir
from concourse._compat import with_exitstack

AF = mybir.ActivationFunctionType
ALU = mybir.AluOpType
DT = mybir.dt

H = W = 48
HP = 24
N = HP * W              # 1152 pixels per core
C = 192
HEADS, D = 12, 16
DG, K, CLIP = 12, 9, 2
M2 = H * (W + 1)        # 2352 pair positions
SCALE = float(D) ** -0.5

TILE_SLOTS = [
    [(0, h) for h in range(8)],
    [(1, h) for h in range(8)],
    [(0, 8), (0, 9), (0, 10), (0, 11), (1, 8), (1, 9), (1, 10), (1, 11)],
]
KY = np.repeat(np.arange(3), 3).astype(np.float32)
KX = np.tile(np.arange(3), 3).astype(np.float32)

F32, BF16, I16 = DT.float32, DT.bfloat16, DT.int16


# ======================================================================
# host prep
# ======================================================================

def host_inputs_for_core(I, b, half):
    r0 = HP * half
    rows = slice(r0, r0 + HP)
    out = {}
    out['qin'] = np.ascontiguousarray(
        np.asarray(I['q'])[b, 0, :, rows, :].reshape(C, N)).astype(np.float32)
    out['kin'] = np.ascontiguousarray(
        np.asarray(I['k'])[b].reshape(CLIP, C, H * W)).astype(np.float32)
    out['vin'] = np.ascontiguousarray(
        np.asarray(I['v'])[b].reshape(CLIP, C, H * W)).astype(np.float32)

    off = np.asarray(I['offset'])[b][:, :, rows, :].reshape(
        CLIP, DG, K, 2, N).astype(np.float32)
    n_row = ((np.arange(N) // W) + r0).astype(np.float32)
    n_col = (np.arange(N) % W).astype(np.float32)

    def pypx(c, g, kk):
        py = off[c, g, kk, 0] + n_row + KY[kk] - np.float32(1.0)
        px = off[c, g, kk, 1] + n_col + KX[kk] - np.float32(1.0)
        return py.astype(np.float32), px.astype(np.float32)

    # k-side weight-pipe input: unique rows (c, gl, kk)
    pyk = np.zeros((108, N), np.float32)
    pxk = np.zeros((108, N), np.float32)
    for c in range(CLIP):
        for gl in range(6):
            for kk in range(K):
                r = kk * 12 + c * 6 + gl
                pyk[r], pxk[r] = pypx(c, gl, kk)
    out['pyj_k'], out['pxj_k'] = pyk, pxk

    # v-side weight-pipe inputs: h-duplicated rows (h, kk), one per clip
    for c in range(CLIP):
        pyv = np.zeros((108, N), np.float32)
        pxv = np.zeros((108, N), np.float32)
        for h in range(HEADS):
            for kk in range(K):
                r = kk * 12 + h
                pyv[r], pxv[r] = pypx(c, 6 + h // 2, kk)
        out[f'pyj_v{c}'], out[f'pxj_v{c}'] = pyv, pxv

    # wrapped index-pipe inputs in gather-slot layout
    pw = np.zeros((2, 3, 2, 128, 648), np.float32)
    for si, side in enumerate(('k', 'v')):
        for t in range(3):
            for s in range(8):
                c, h = TILE_SLOTS[t][s]
                g = (h // 2) + (6 if side == 'v' else 0)
                for kk in range(K):
                    py, px = pypx(c, g, kk)
                    pw[si, t, 0, 16 * s:16 * s + 16,
                       72 * kk:72 * (kk + 1)] = py.reshape(72, 16).T
                    pw[si, t, 1, 16 * s:16 * s + 16,
                       72 * kk:72 * (kk + 1)] = px.reshape(72, 16).T
    out['pw'] = np.ascontiguousarray(pw)

    for nm, w in (('wqT', I['wq']), ('wkT', I['wk']), ('wvT', I['wv']),
                  ('w1T', I['w1']), ('w2T', I['w2'])):
        out[nm] = np.ascontiguousarray(np.asarray(w).T.astype(np.float32))
    for nm in ('bq', 'bk', 'bv', 'b1', 'b2'):
        out[nm] = np.asarray(I[nm]).astype(np.float32).reshape(-1, 1)

    blk32 = np.zeros((128, 32), np.float32)
    for s in range(8):
        blk32[16 * s:16 * s + 16, s] = 1.0
    out['blk32'] = blk32
    out['id128'] = np.eye(128, dtype=np.float32)
    zselh = np.zeros((108, 12), np.float32)
    for h in range(HEADS):
        for kk in range(K):
            zselh[kk * 12 + h, h] = 1.0
    out['zselh'] = zselh
    sum2 = np.zeros((128, 64), np.float32)
    for j in range(64):
        sum2[j, j] = 1.0
        sum2[64 + j, j] = 1.0
    out['sum2'] = sum2
    return out


INPUT_SPECS = {
    'qin': (C, N), 'kin': (CLIP, C, H * W), 'vin': (CLIP, C, H * W),
    'pyj_k': (108, N), 'pxj_k': (108, N),
    'pyj_v0': (108, N), 'pxj_v0': (108, N),
    'pyj_v1': (108, N), 'pxj_v1': (108, N),
    'pw': (2, 3, 2, 128, 648),
    'wqT': (C, C), 'wkT': (C, C), 'wvT': (C, C),
    'w1T': (C, 2 * C), 'w2T': (2 * C, C),
    'bq': (C, 1), 'bk': (C, 1), 'bv': (C, 1),
    'b1': (2 * C, 1), 'b2': (C, 1),
    'blk32': (128, 32), 'id128': (128, 128),
    'zselh': (108, 12), 'sum2': (128, 64),
}


# ======================================================================
# device program
# ======================================================================

@with_exitstack
def device_kernel(ctx: ExitStack, tc: tile.TileContext, outs, ins):
    nc = tc.nc
    out_dram = outs['out']

    pool = ctx.enter_context(tc.tile_pool(name="persist", bufs=1))
    pool3 = ctx.enter_context(tc.tile_pool(name="work", bufs=2))

    dma = nc.sync.dma_start
    dma2 = nc.scalar.dma_start
    dma3 = nc.sync.dma_start

    def loadt(name, dt=F32, eng=dma, pl=None, tag=None):
        shp = INPUT_SPECS[name]
        src = ins[name]
        if len(shp) > 2:
            dims = " ".join(f"a{i}" for i in range(len(shp)))
            outer = " ".join(f"a{i}" for i in range(len(shp) - 1))
            src = src.rearrange(f"{dims} -> ({outer}) a{len(shp) - 1}")
        parts, fr = int(np.prod(shp[:-1])), shp[-1]
        tiles = []
        p0 = 0
        while p0 < parts:
            p = min(128, parts - p0)
            t = (pl or pool).tile([p, fr], dt, tag=(tag or name) + f"_{p0}",
                                  name=(tag or name) + f"_{p0}")
            eng(t[:], src[p0:p0 + p, :])
            tiles.append(t)
            p0 += p
        return tiles

    wq_t = loadt('wqT'); wk_t = loadt('wkT'); wv_t = loadt('wvT')
    w1_t = loadt('w1T'); w2_t = loadt('w2T')
    bq_t = loadt('bq'); bk_t = loadt('bk'); bv_t = loadt('bv')
    b1_t = loadt('b1'); b2_t = loadt('b2')
    sum2_t = loadt('sum2')[0]

    def as_bf16(src_tiles, tag):
        t0 = src_tiles[0]
        t = pool.tile(list(t0[:].shape), BF16, tag=tag)
        nc.vector.tensor_copy(t[:], t0[:])
        return t

    blk32_bf = as_bf16(loadt('blk32'), "blk32b")
    id_bf = as_bf16(loadt('id128'), "id128b")
    zselh_bf = as_bf16(loadt('zselh'), "zselhb")

    kin, vin = ins['kin'], ins['vin']

    # ---------------- projections -> pair sources ----------------
    NCHW = 384

    def make_src():
        ts = [pool.tile([128, M2, 2], BF16, tag=f"src_{t}", name=f"src{t}")
          for t in range(3)]
        for t in ts:
            nc.vector.memset(t[:], 0.0)
        return ts

    def project_to_src(x_dram, w_tiles, b_tiles, src_tiles, psp, tmp_tag):
        for c in range(CLIP):
            for co0, co_p in ((0, 128), (128, 64)):
                for ncx in range(H // 8):
                    ps = psp.tile([co_p, NCHW], F32, tag="proj_ps")
                    xa = pool3.tile([128, NCHW], F32, tag="proj_in_a")
                    xb = pool3.tile([64, NCHW], F32, tag="proj_in_b")
                    dma(xa[:], x_dram[c, 0:128, ncx * NCHW:(ncx + 1) * NCHW])
                    dma(xb[:], x_dram[c, 128:192, ncx * NCHW:(ncx + 1) * NCHW])
                    nc.tensor.matmul(ps[:], w_tiles[0][:, co0:co0 + co_p],
                                     xa[:], start=True, stop=False)
                    nc.tensor.matmul(ps[:], w_tiles[1][:, co0:co0 + co_p],
                                     xb[:], start=False, stop=True)
                    bias = b_tiles[0 if co0 == 0 else 1]
                    y0 = 8 * ncx
                    psv = ps[:].rearrange("p (y x) -> p y x", y=8)
                    if co0 == 0 or c == 0:
                        dst = src_tiles[c] if co0 == 0 else src_tiles[2]
                        sl = dst[0:co_p, y0 * 49:(y0 + 8) * 49, :].rearrange(
                            "p (y x) l -> p y x l", y=8)
                        nc.vector.tensor_scalar(
                            sl[:, :, 0:48, 1], psv, bias[:], None, ALU.add)
                        nc.vector.tensor_scalar(
                            sl[:, :, 1:49, 0], psv, bias[:], None, ALU.add)
                    else:
                        tmp = pool3.tile([64, 8 * 49, 2], BF16,
                                         tag=f"t2tmp_{tmp_tag}")
                        nc.vector.memset(tmp[:], 0.0)
                        tv = tmp[:].rearrange("p (y x) l -> p y x l", y=8)
                        nc.vector.tensor_scalar(
                            tv[:, :, 0:48, 1], psv, bias[:], None, ALU.add)
                        nc.vector.tensor_scalar(
                            tv[:, :, 1:49, 0], psv, bias[:], None, ALU.add)
                        dma(src_tiles[2][64:128, y0 * 49:(y0 + 8) * 49, :],
                            tmp[:])

    # q projection -> q2 lane-duplicated bf16
    q2 = {0: pool.tile([128, N, 2], BF16, tag="q2_0", name="q2a"),
          2: pool.tile([128, N, 2], BF16, tag="q2_2", name="q2b")}
    qp_tail = pool.tile([64, N], BF16, tag="qp_tail")
    with tc.tile_pool(name="psq", bufs=2, space="PSUM") as psq, \
            tc.tile_pool(name="qpool", bufs=1) as qpl:
        qin_t = loadt('qin', pl=qpl)
        for co0, co_p in ((0, 128), (128, 64)):
            for ncx in range(3):
                nw = 384
                ps = psq.tile([co_p, nw], F32, tag="q_ps")
                nc.tensor.matmul(ps[:], wq_t[0][:, co0:co0 + co_p],
                                 qin_t[0][:, ncx * nw:(ncx + 1) * nw],
                                 start=True, stop=False)
                nc.tensor.matmul(ps[:], wq_t[1][:, co0:co0 + co_p],
                                 qin_t[1][:, ncx * nw:(ncx + 1) * nw],
                                 start=False, stop=True)
                if co0 == 0:
                    for lane in range(2):
                        nc.vector.tensor_scalar(
                            q2[0][:, ncx * nw:(ncx + 1) * nw, lane],
                            ps[:], bq_t[0][:], SCALE, ALU.add, ALU.mult)
                else:
                    nc.vector.tensor_scalar(
                        qp_tail[:, ncx * nw:(ncx + 1) * nw],
                        ps[:], bq_t[1][:], SCALE, ALU.add, ALU.mult)
        q2t2s = qpl.tile([128, N], BF16, tag="q2t2s")
        dma(q2t2s[0:64, :], qp_tail[:])
        dma(q2t2s[64:128, :], qp_tail[:])
        for lane in range(2):
            nc.scalar.copy(q2[2][:, :, lane], q2t2s[:])

    ksrc = make_src()
    with tc.tile_pool(name="psk", bufs=2, space="PSUM") as psk:
        project_to_src(kin, wk_t, bk_t, ksrc, psk, 'k')

    # ---------------- weight pipelines ----------------
    def wpipe(wp, outpool, pyj, pxj, tag):
        def axis(pj, hi, sfx):
            pc = wp.tile([108, N], F32, tag="wp_A")
            nc.vector.tensor_scalar(pc[:], pj[:], -2.0, 49.0, ALU.max, ALU.min)
            c16 = wp.tile([108, N], I16, tag="wp_B")
            nc.vector.tensor_copy(c16[:], pc[:])
            cf = wp.tile([108, N], F32, tag="wp_C")
            nc.vector.tensor_copy(cf[:], c16[:])
            gt = wp.tile([108, N], F32, tag="wp_D")
            nc.any.tensor_tensor(gt[:], cf[:], pc[:], ALU.is_gt)
            p0 = cf
            nc.vector.tensor_sub(p0[:], cf[:], gt[:])
            f = wp.tile([108, N], F32, tag="wp_F")
            nc.vector.tensor_sub(f[:], pc[:], p0[:])
            p0c = gt
            nc.vector.tensor_scalar(p0c[:], p0[:], 0.0, hi, ALU.max, ALU.min)
            v0 = wp.tile([108, N], F32, tag="wp_G")
            nc.any.tensor_tensor(v0[:], p0[:], p0c[:], ALU.is_equal)
            w0 = p0c
            nc.scalar.activation(w0[:], f[:], AF.Copy, bias=1.0, scale=-1.0)
            w0e = wp.tile([108, N], F32, tag="wp_w0e" + sfx)
            nc.vector.tensor_mul(w0e[:], w0[:], v0[:])
            p1 = v0
            nc.scalar.activation(p1[:], p0[:], AF.Copy, bias=1.0, scale=1.0)
            p1c = w0
            nc.vector.tensor_scalar(p1c[:], p1[:], 0.0, hi, ALU.max, ALU.min)
            v1 = p0
            nc.any.tensor_tensor(v1[:], p1[:], p1c[:], ALU.is_equal)
            w1e = wp.tile([108, N], F32, tag="wp_w1e" + sfx)
            nc.vector.tensor_mul(w1e[:], f[:], v1[:])
            return w0e, w1e

        wy0e, wy1e = axis(pyj, 47.0, "y")
        wx0e, wx1e = axis(pxj, 47.0, "x")
        wtop = outpool.tile([108, N, 2], BF16, tag=f"wtop_{tag}",
                            name=f"wtop{tag}")
        wbot = outpool.tile([108, N, 2], BF16, tag=f"wbot_{tag}",
                            name=f"wbot{tag}")
        nc.vector.tensor_mul(wtop[:, :, 0], wy0e[:], wx0e[:])
        nc.vector.tensor_mul(wtop[:, :, 1], wy0e[:], wx1e[:])
        nc.vector.tensor_mul(wbot[:, :, 0], wy1e[:], wx0e[:])
        nc.vector.tensor_mul(wbot[:, :, 1], wy1e[:], wx1e[:])
        return wtop, wbot

    kwt = ctx.enter_context(tc.tile_pool(name="kwt", bufs=1))
    with tc.tile_pool(name="wpool", bufs=1) as wp:
        pyk_t = loadt('pyj_k', pl=wp)[0]
        pxk_t = loadt('pxj_k', pl=wp)[0]
        wtop_k, wbot_k = wpipe(wp, kwt, pyk_t, pxk_t, 'k')

    # ---------------- index pipelines ----------------
    gidx = {}
    pw_flat = ins['pw'].rearrange("a b c d e -> (a b c d) e")
    with tc.tile_pool(name="ipool", bufs=1) as ip:
        for si, side in enumerate(('k', 'v')):
            for t in range(3):
                pyw = pool3.tile([128, 648], F32, tag="pw_y")
                pxw = pool3.tile([128, 648], F32, tag="pw_x")
                r0_ = ((si * 3 + t) * 2) * 128
                dma(pyw[:], pw_flat[r0_:r0_ + 128, :])
                dma(pxw[:], pw_flat[r0_ + 128:r0_ + 256, :])

                def iax(pwt):
                    pc = ip.tile([128, 648], F32, tag="ip_pc")
                    nc.vector.tensor_scalar(pc[:], pwt[:], -2.0, 49.0,
                                            ALU.max, ALU.min)
                    c16 = ip.tile([128, 648], I16, tag="ip_c16")
                    nc.vector.tensor_copy(c16[:], pc[:])
                    cf = ip.tile([128, 648], F32, tag="ip_cf")
                    nc.vector.tensor_copy(cf[:], c16[:])
                    gt = ip.tile([128, 648], F32, tag="ip_gt")
                    nc.any.tensor_tensor(gt[:], cf[:], pc[:], ALU.is_gt)
                    p0 = ip.tile([128, 648], F32, tag="ip_p0")
                    nc.vector.tensor_sub(p0[:], cf[:], gt[:])
                    return p0

                y0 = iax(pyw)
                y0c = ip.tile([128, 648], F32, tag="ip_y0c")
                nc.vector.tensor_scalar(y0c[:], y0[:], 0.0, 47.0,
                                        ALU.max, ALU.min)
                y1c = ip.tile([128, 648], F32, tag="ip_y1c")
                nc.vector.tensor_scalar(y1c[:], y0[:], 1.0, 0.0,
                                        ALU.add, ALU.max)
                nc.vector.tensor_scalar(y1c[:], y1c[:], 47.0, None, ALU.min)
                x0 = iax(pxw)
                jx = ip.tile([128, 648], F32, tag="ip_jx")
                nc.vector.tensor_scalar(jx[:], x0[:], 1.0, 0.0,
                                        ALU.add, ALU.max)
                nc.vector.tensor_scalar(jx[:], jx[:], 48.0, None, ALU.min)
                itf = ip.tile([128, 648], F32, tag="ip_itf")
                nc.vector.tensor_scalar(itf[:], y0c[:], 49.0, None, ALU.mult)
                nc.vector.tensor_add(itf[:], itf[:], jx[:])
                ibf = ip.tile([128, 648], F32, tag="ip_ibf")
                nc.vector.tensor_scalar(ibf[:], y1c[:], 49.0, None, ALU.mult)
                nc.vector.tensor_add(ibf[:], ibf[:], jx[:])
                it16 = pool.tile([128, 648], I16, tag=f"it16_{side}{t}")
                ib16 = pool.tile([128, 648], I16, tag=f"ib16_{side}{t}")
                nc.vector.tensor_copy(it16[:], itf[:])
                nc.vector.tensor_copy(ib16[:], ibf[:])
                gidx[(side, t, 't')] = it16
                gidx[(side, t, 'b')] = ib16

    # ---------------- k-wave ----------------
    attn = [pool.tile([108, N], BF16, tag=f"attn_{c}", name=f"attn{c}")
            for c in range(CLIP)]

    # k-wave proper
    with tc.tile_pool(name="pskw", bufs=1, space="PSUM") as pskw, \
            tc.tile_pool(name="kwork", bufs=2) as kw:
        for t in range(3):
            for kg in range(3):
                kks = [3 * kg, 3 * kg + 1, 3 * kg + 2]
                # weight tiles in psum-row layout [96, N, 2]
                wrep = {}
                for row, wsrc in (('t', wtop_k), ('b', wbot_k)):
                    wr = kw.tile([96, N, 2], BF16, tag="kg_wr")
                    nc.vector.memset(wr[:], 0.0)
                    for ki, kk in enumerate(kks):
                        if t < 2:
                            c0 = TILE_SLOTS[t][0][0]
                            base = kk * 12 + c0 * 6
                            src = wsrc[base:base + 4, :, :].unsqueeze(
                                1).broadcast_to([4, 2, N, 2])
                            dma2(wr[32 * ki:32 * ki + 8, :, :], src)
                        else:
                            for ch in range(2):
                                base = kk * 12 + ch * 6 + 4
                                src = wsrc[base:base + 2, :, :].unsqueeze(
                                    1).broadcast_to([2, 2, N, 2])
                                dma2(wr[32 * ki + 4 * ch:
                                        32 * ki + 4 * ch + 4, :, :], src)
                    wrep[row] = wr
                sc = kw.tile([96, N], F32, tag="kg_sc")
                for row in ('t', 'b'):
                    ps = pskw.tile([96, N, 2], F32, tag="dots")
                    for ki, kk in enumerate(kks):
                        g = kw.tile([128, N, 2], BF16, tag="kg_g")
                        nc.gpsimd.ap_gather(
                            g[:], ksrc[t][:],
                            gidx[('k', t, row)][:, 72 * kk:72 * (kk + 1)],
                            channels=128, num_elems=M2, d=2, num_idxs=N)
                        p = kw.tile([128, N, 2], BF16, tag="kg_p")
                        nc.vector.tensor_mul(p[:], g[:],
                                             q2[0 if t < 2 else 2][:])
                        pr = p[:].rearrange("p n l -> p (n l)")
                        pf = ps[:].rearrange("p n l -> p (n l)")
                        for c0 in range(0, 2304, 512):
                            cw = min(512, 2304 - c0)
                            nc.tensor.matmul(
                                pf[32 * ki:32 * ki + 32, c0:c0 + cw],
                                blk32_bf[:], pr[:, c0:c0 + cw],
                                start=True, stop=True,
                                tile_position=(0, 32 * ki))
                    e = kw.tile([96, N, 2], F32, tag="kg_e", bufs=1)
                    nc.any.tensor_tensor(e[:], ps[:], wrep[row][:], ALU.mult)
                    if row == 't':
                        nc.vector.tensor_add(sc[:], e[:, :, 0], e[:, :, 1])
                    else:
                        nc.vector.tensor_add(sc[:], sc[:], e[:, :, 0])
                        nc.vector.tensor_add(sc[:], sc[:], e[:, :, 1])
                esc = kw.tile([96, N], BF16, tag="kg_esc")
                nc.scalar.activation(esc[:], sc[:], AF.Exp)
                # attn scatter
                for ki, kk in enumerate(kks):
                    if t < 2:
                        c0 = TILE_SLOTS[t][0][0]
                        dma3(attn[c0][kk * 12:kk * 12 + 8, :],
                             esc[32 * ki:32 * ki + 8, :])
                    else:
                        for ch in range(2):
                            dma3(attn[ch][kk * 12 + 8:kk * 12 + 12, :],
                                 esc[32 * ki + 4 * ch:
                                     32 * ki + 4 * ch + 4, :])

    # ---------------- Z from attn tiles (PE) ----------------
    zsum = pool.tile([HEADS, N], F32, tag="zsum")
    with tc.tile_pool(name="psz", bufs=1, space="PSUM") as psz:
        zp = psz.tile([HEADS, N], F32, tag="zp")
        for ci in range(CLIP):
            for c0 in range(0, N, 512):
                cw = min(512, N - c0)
                nc.tensor.matmul(zp[:, c0:c0 + cw], zselh_bf[:],
                                 attn[ci][:, c0:c0 + cw],
                                 start=(ci == 0), stop=(ci == 1))
        nc.vector.reciprocal(zsum[:], zp[:])

    # ---------------- v-wave ----------------
    vsrc = make_src()
    with tc.tile_pool(name="psv", bufs=2, space="PSUM") as psv:
        project_to_src(vin, wv_t, bv_t, vsrc, psv, 'v')

    wtop_v, wbot_v = {}, {}
    with tc.tile_pool(name="wpool2", bufs=1) as wp2:
        for c in range(CLIP):
            pyv_t = loadt(f'pyj_v{c}', pl=wp2)[0]
            pxv_t = loadt(f'pxj_v{c}', pl=wp2)[0]
            wtop_v[c], wbot_v[c] = wpipe(wp2, pool, pyv_t, pxv_t,
                                         f'v{c}')

    aw = {}
    for c in range(CLIP):
        for row, wsrc in (('t', wtop_v), ('b', wbot_v)):
            a2 = wsrc[c]
            for lane in range(2):
                nc.vector.tensor_mul(a2[:, :, lane], attn[c][:],
                                     a2[:, :, lane])
            aw[(c, row)] = a2

    tail = ctx.enter_context(tc.tile_pool(name="tail", bufs=1))
    zr_main = tail.tile([128, N], F32, tag="zr_main")
    dma2(zr_main[:, :], zsum[0:8, :].unsqueeze(1).broadcast_to([8, 16, N]))
    zr_tail = tail.tile([64, N], F32, tag="zr_tail")
    dma2(zr_tail[:, :], zsum[8:12, :].unsqueeze(1).broadcast_to([4, 16, N]))

    o_main = tail.tile([128, N], F32, tag="o_main")
    t2sb = tail.tile([128, N], F32, tag="t2sb")
    with tc.tile_pool(name="psacc", bufs=1, space="PSUM") as psacc, \
            tc.tile_pool(name="vwork", bufs=2) as vw:
        acc_main = psacc.tile([128, N], F32, tag="acc_main")
        acc_t2 = psacc.tile([128, N], F32, tag="acc_t2")
        nmm = [0, 0]
        for t in range(3):
            for kk in range(K):
                for row in ('t', 'b'):
                    g = vw.tile([128, N, 2], BF16, tag="vg_g")
                    nc.gpsimd.ap_gather(
                        g[:], vsrc[t][:],
                        gidx[('v', t, row)][:, 72 * kk:72 * (kk + 1)],
                        channels=128, num_elems=M2, d=2, num_idxs=N)
                    wr = vw.tile([128, N, 2], BF16, tag="vg_wr")
                    if t < 2:
                        c0 = TILE_SLOTS[t][0][0]
                        src = aw[(c0, row)][kk * 12:kk * 12 + 8, :, :]
                        src = src.unsqueeze(1).broadcast_to([8, 16, N, 2])
                        dma2(wr[:, :, :], src)
                    else:
                        for ch in range(2):
                            src = aw[(ch, row)][kk * 12 + 8:kk * 12 + 12,
                                                :, :]
                            src = src.unsqueeze(1).broadcast_to(
                                [4, 16, N, 2])
                            dma2(wr[64 * ch:64 * ch + 64, :, :], src)
                    m = vw.tile([128, N, 2], BF16, tag="vg_m")
                    nc.vector.tensor_mul(m[:], g[:], wr[:])
                    ms = vw.tile([128, N], BF16, tag="vg_ms")
                    nc.vector.tensor_add(ms[:], m[:, :, 0], m[:, :, 1])
                    zi = 0 if t < 2 else 1
                    accp = acc_main if t < 2 else acc_t2
                    nmm[zi] += 1
                    last = (nmm[zi] == (2 * K * 2 if zi == 0 else K * 2))
                    for c0 in range(0, N, 512):
                        cw = min(512, N - c0)
                        nc.tensor.matmul(accp[:, c0:c0 + cw], id_bf[:],
                                         ms[:, c0:c0 + cw],
                                         start=(nmm[zi] == 1), stop=last)
        nc.vector.tensor_mul(o_main[:], acc_main[:], zr_main[:])
        nc.vector.tensor_copy(t2sb[:], acc_t2[:])

    # ---------------- tail sum + MLP ----------------
    with tc.tile_pool(name="psmlp", bufs=2, space="PSUM") as psm:
        o_tail = tail.tile([64, N], F32, tag="o_tail")
        for c0 in range(0, N, 384):
            ps = psm.tile([64, 384], F32, tag="t2_ps")
            nc.tensor.matmul(ps[:], sum2_t[:, 0:64], t2sb[:, c0:c0 + 384],
                             start=True, stop=True)
            nc.vector.tensor_mul(o_tail[:, c0:c0 + 384], ps[:],
                                  zr_tail[:, c0:c0 + 384])

        h1 = [tail.tile([128, N], F32, tag=f"h1_{i}", name=f"h1{i}")
              for i in range(3)]
        for i in range(3):
            for ncx in range(3):
                nw = 384
                ps = psm.tile([128, nw], F32, tag="mlp_ps")
                nc.tensor.matmul(ps[:], w1_t[0][:, 128 * i:128 * (i + 1)],
                                 o_main[:, ncx * nw:(ncx + 1) * nw],
                                 start=True, stop=False)
                nc.tensor.matmul(ps[:], w1_t[1][:, 128 * i:128 * (i + 1)],
                                 o_tail[:, ncx * nw:(ncx + 1) * nw],
                                 start=False, stop=True)
                nc.scalar.activation(h1[i][:, ncx * nw:(ncx + 1) * nw],
                                     ps[:], AF.Gelu, bias=b1_t[i][:])
        out_sb = [tail.tile([128, N], F32, tag="out0", name="outsb0"),
                  tail.tile([64, N], F32, tag="out1", name="outsb1")]
        for i, (co0, co_p) in enumerate(((0, 128), (128, 64))):
            for ncx in range(3):
                nw = 384
                ps = psm.tile([co_p, nw], F32, tag="mlp_ps2")
                for j in range(3):
                    nc.tensor.matmul(ps[:], w2_t[j][:, co0:co0 + co_p],
                                     h1[j][:, ncx * nw:(ncx + 1) * nw],
                                     start=(j == 0), stop=(j == 2))
                osrc = o_main if i == 0 else o_tail
                bias = b2_t[0] if i == 0 else b2_t[1]
                tmp = pool3.tile([co_p, nw], F32, tag="mlp_tmp")
                nc.vector.tensor_scalar(tmp[:], ps[:], bias[:], None, ALU.add)
                nc.vector.tensor_add(out_sb[i][:, ncx * nw:(ncx + 1) * nw],
                                     tmp[:], osrc[:, ncx * nw:(ncx + 1) * nw])
    dma(out_dram[0:128, :], out_sb[0][:])
    dma(out_dram[128:192, :], out_sb[1][:])


# ======================================================================
# launch
# ======================================================================

def _build_program():
    import concourse.bacc as bacc
    nc = bacc.Bacc("TRN2", target_bir_lowering=False, debug=False,
                   num_devices=8)
    in_aps = {}
    for name, shp in INPUT_SPECS.items():
        in_aps[name] = nc.dram_tensor(
            name, list(shp), F32, kind="ExternalInput").ap()
    out_ap = nc.dram_tensor("out", [C, N], F32, kind="ExternalOutput").ap()
    with tile.TileContext(nc) as t:
        device_kernel(t, {'out': out_ap}, in_aps)
    nc.compile()
    return nc


_PROGRAM = None


def kernel(**inputs):
    global _PROGRAM
    from concourse import bass_utils
    if _PROGRAM is None:
        _PROGRAM = _build_program()
    in_maps = []
    for core in range(8):
        b, half = core // 2, core % 2
        in_maps.append(host_inputs_for_core(inputs, b, half))
    res = bass_utils.run_bass_kernel_spmd(
        _PROGRAM, in_maps, core_ids=list(range(8)))
    out = np.zeros((4, 1, C, H, W), np.float32)
    for core in range(8):
        b, half = core // 2, core % 2
        o = res.results[core]['out'].reshape(C, HP, W)
        out[b, 0, :, HP * half:HP * (half + 1), :] = o
    return out

